# revision 8
# baseline (speedup 1.0000x reference)
"""Cross-attention kernel for 8 TRN2 NeuronCores.

Reference computation (per problem spec):
    q = (x @ Wq)  [B=4, N=4096, D=1024] -> heads [B, 16, N, 64]
    k = (context @ Wk), v = (context @ Wv)   context [B, M=256, 768]
    out = softmax(q k^T / 8 + mask) v   -> [B, N, D] @ Wo

Sharding: the 16384 query rows (B*N) are split evenly across the 8 cores
(2048 rows each, each shard living inside one batch). K/V are computed
redundantly per core from that core's batch context (only ~0.8 GFLOP) so no
collectives are needed; each core produces its own 2048 output rows and the
host concatenates them.

On-core layout strategy ("transposed" dataflow, bf16 matmul operands with
fp32 PSUM accumulation):
  x^T (PE transpose) -> Q^T = Wq^T x^T -> scores^T = K Q^T (per head, kv on
  partitions, two heads row-packed in the PE array) -> exp via ScalarE with
  the context-mask as per-partition bias -> O^T = V^T attn^T (two heads
  col-packed, separate PSUM banks) with the softmax denominator obtained by
  an all-ones stationary matmul broadcast to the head's partitions ->
  normalize on PSUM eviction -> out = O^T.T Wo.
"""

import sys

for _p in ("/opt/trn_rl_repo",):
    if _p not in sys.path:
        sys.path.insert(0, _p)

import numpy as np

import concourse.bass as bass
import concourse.mybir as mybir
import concourse.tile as tile
from concourse import bacc
from concourse.bass_utils import run_bass_kernel_spmd
from concourse.masks import make_identity

ts = bass.ts

N_CORES = 8
B, N, D = 4, 4096, 1024
CTX = 768
M = 256          # kv length
H, HD = 16, 64   # heads, head dim
NQ = (B * N) // N_CORES   # 2048 query rows per core
QCH = 512                 # q chunk (free dim of most matmuls)
NQC = NQ // QCH           # 4 q chunks
DT = D // 128             # 8 d-blocks (= head pairs)
KCH = CTX // 128          # 6 contraction chunks for context projections
F32 = mybir.dt.float32
BF16 = mybir.dt.bfloat16

SCALE = HD ** -0.5


def build_nc():
    nc = bacc.Bacc()

    x_ext = nc.declare_dram_parameter("x", [NQ, D], F32, isOutput=False)
    ctx_ext = nc.declare_dram_parameter("ctx", [M, CTX], F32, isOutput=False)
    maskb_ext = nc.declare_dram_parameter("maskb", [128, 2], F32, isOutput=False)
    wq_ext = nc.declare_dram_parameter("wq", [D, D], F32, isOutput=False)
    wk_ext = nc.declare_dram_parameter("wk", [CTX, D], F32, isOutput=False)
    wv_ext = nc.declare_dram_parameter("wv", [CTX, D], F32, isOutput=False)
    wo_ext = nc.declare_dram_parameter("wo", [D, D], F32, isOutput=False)
    out_ext = nc.declare_dram_parameter("out", [NQ, D], F32, isOutput=True)

    with tile.TileContext(nc) as tc:
        # ---- persistent tensors -------------------------------------------
        ident, free_ident = tc.tile([128, 128], BF16, name="ident")
        make_identity(nc, ident)
        ones_sb, free_ones = tc.tile([128, HD], BF16, name="ones_sb")
        nc.vector.memset(ones_sb, 1.0)
        mask_sb, free_mask = tc.tile([128, 2], F32, name="mask_sb")
        nc.sync.dma_start(out=mask_sb, in_=maskb_ext[:, :])

        kT, free_kT = tc.tile([128, DT, M], BF16, name="kT")
        vv, free_vv = tc.tile([128, 2, D], BF16, name="vv")
        xT, free_xT = tc.tile([128, DT, NQ], BF16, name="xT")
        qT, free_qT = tc.tile([128, DT, NQ], BF16, name="qT", side="right")

        # ---- phase B: context -> ctx^T -> K^T, V --------------------------
        with tc.tile_pool(name="bpool", bufs=1) as bpool, \
             tc.tile_pool(name="bpsum", bufs=2, space="PSUM") as bpsum:
            ctx_sb = bpool.tile([128, 2, CTX], BF16, name="ctx_sb")
            nc.gpsimd.dma_start(
                out=ctx_sb, in_=ctx_ext.rearrange("(a p) n -> p a n", p=128)
            )
            wk_sb = bpool.tile([128, KCH, D], BF16, name="wk_sb")
            nc.gpsimd.dma_start(
                out=wk_sb, in_=wk_ext.rearrange("(a p) n -> p a n", p=128)
            )
            wv_sb = bpool.tile([128, KCH, D], BF16, name="wv_sb")
            nc.gpsimd.dma_start(
                out=wv_sb, in_=wv_ext.rearrange("(a p) n -> p a n", p=128)
            )
            ctxT = bpool.tile([128, KCH, M], BF16, name="ctxT")
            for a in range(2):  # kv row blocks
                tp = bpsum.tile([128, KCH, 128], BF16, name="tp", tag="tp")
                for k in range(KCH):
                    nc.tensor.transpose(
                        tp[:, k, :], ctx_sb[:, a, ts(k, 128)], ident
                    )
                nc.vector.tensor_copy(ctxT[:, :, ts(a, 128)], tp)
            # K^T [d-block, kv]
            for m in range(DT):
                ps = bpsum.tile([128, M], F32, name="ps", tag="ps")
                for k in range(KCH):
                    nc.tensor.matmul(
                        ps[:, :], wk_sb[:, k, ts(m, 128)], ctxT[:, k, :],
                        start=(k == 0), stop=(k == KCH - 1),
                    )
                nc.vector.tensor_copy(kT[:, m, :], ps)
            # V [kv-part, D]
            for j in range(2):
                for n in range(2):
                    psv = bpsum.tile([128, 512], F32, name="psv", tag="psv")
                    for k in range(KCH):
                        nc.tensor.matmul(
                            psv[:, :], ctxT[:, k, ts(j, 128)],
                            wv_sb[:, k, ts(n, 512)],
                            start=(k == 0), stop=(k == KCH - 1),
                        )
                    nc.vector.tensor_copy(vv[:, j, ts(n, 512)], psv)

        # ---- phase A: x -> x^T -> Q^T -------------------------------------
        with tc.tile_pool(name="apool", bufs=3) as apool, \
             tc.tile_pool(name="awq", bufs=1) as awq, \
             tc.tile_pool(name="apsum", bufs=4, space="PSUM") as apsum:
            wq_sb = awq.tile([128, DT, D], BF16, name="wq_sb")
            nc.gpsimd.dma_start(
                out=wq_sb, in_=wq_ext.rearrange("(a p) n -> p a n", p=128)
            )
            for rb in range(NQ // 128):  # 16 row blocks
                x_sb = apool.tile([128, D], BF16, name="x_sb", tag="x_sb")
                nc.gpsimd.dma_start(out=x_sb, in_=x_ext[ts(rb, 128), :])
                for kg in range(2):  # groups of 4 d-blocks
                    tp = apsum.tile([128, 4, 128], BF16, name="tp", tag="tp")
                    for kk in range(4):
                        nc.tensor.transpose(
                            tp[:, kk, :], x_sb[:, ts(4 * kg + kk, 128)], ident
                        )
                    nc.vector.tensor_copy(
                        xT[:, 4 * kg : 4 * kg + 4, ts(rb, 128)], tp
                    )
            for m in range(DT):
                for c in range(NQC):
                    ps = apsum.tile([128, QCH], F32, name="ps", tag="ps")
                    for k in range(DT):
                        nc.tensor.matmul(
                            ps[:, :], wq_sb[:, k, ts(m, 128)],
                            xT[:, k, ts(c, QCH)],
                            start=(k == 0), stop=(k == DT - 1),
                        )
                    nc.vector.tensor_copy(qT[:, m, ts(c, QCH)], ps)
        free_xT()

        # ---- phase C: attention per head pair -----------------------------
        oT, free_oT = tc.tile([128, DT, NQ], BF16, name="oT")
        with tc.tile_pool(name="cattn", bufs=6) as cattn, \
             tc.tile_pool(name="crec", bufs=3) as crec, \
             tc.tile_pool(name="csc", bufs=2, space="PSUM") as csc, \
             tc.tile_pool(name="cav", bufs=1, space="PSUM") as cav, \
             tc.tile_pool(name="csum", bufs=1, space="PSUM") as csum:
            for i in range(DT):       # head pair (2i, 2i+1)
                for c in range(NQC):  # q chunk
                    attns = []
                    for j in range(2):  # kv chunk
                        sc_h = csc.tile([128, QCH], F32, name="sc_h", tag="sc_h")
                        sc_p = csc.tile([128, QCH], F32, name="sc_p", tag="sc_p")
                        nc.tensor.matmul(
                            sc_h[:, :], kT[0:64, i, ts(j, 128)],
                            qT[0:64, i, ts(c, QCH)],
                            start=True, stop=True, tile_position=(0, 0),
                        )
                        nc.tensor.matmul(
                            sc_p[:, :], kT[64:128, i, ts(j, 128)],
                            qT[64:128, i, ts(c, QCH)],
                            start=True, stop=True, tile_position=(64, 0),
                        )
                        at_h = cattn.tile([128, QCH], BF16, name="at_h", tag="at_h")
                        at_p = cattn.tile([128, QCH], BF16, name="at_p", tag="at_p")
                        nc.scalar.activation(
                            at_h, sc_h, mybir.ActivationFunctionType.Exp,
                            bias=mask_sb[:, j : j + 1], scale=SCALE,
                        )
                        nc.scalar.activation(
                            at_p, sc_p, mybir.ActivationFunctionType.Exp,
                            bias=mask_sb[:, j : j + 1], scale=SCALE,
                        )
                        attns.append((at_h, at_p))
                    av_h = cav.tile([128, QCH], F32, name="av_h", tag="av_h")
                    av_p = cav.tile([128, QCH], F32, name="av_p", tag="av_p")
                    sm_h = csum.tile([128, QCH], F32, name="sm_h", tag="sm_h")
                    sm_p = csum.tile([128, QCH], F32, name="sm_p", tag="sm_p")
                    for j in range(2):
                        at_h, at_p = attns[j]
                        nc.tensor.matmul(
                            av_h[0:64, :], vv[:, j, ts(2 * i, HD)], at_h,
                            start=(j == 0), stop=(j == 1), tile_position=(0, 0),
                        )
                        nc.tensor.matmul(
                            av_p[64:128, :], vv[:, j, ts(2 * i + 1, HD)], at_p,
                            start=(j == 0), stop=(j == 1), tile_position=(0, 64),
                        )
                        nc.tensor.matmul(
                            sm_h[0:64, :], ones_sb[:, :], at_h,
                            start=(j == 0), stop=(j == 1), tile_position=(0, 0),
                        )
                        nc.tensor.matmul(
                            sm_p[64:128, :], ones_sb[:, :], at_p,
                            start=(j == 0), stop=(j == 1), tile_position=(0, 64),
                        )
                    rec = crec.tile([128, QCH], F32, name="rec", tag="rec")
                    nc.vector.reciprocal(rec[0:64, :], sm_h[0:64, :])
                    nc.vector.reciprocal(rec[64:128, :], sm_p[64:128, :])
                    nc.vector.tensor_mul(
                        oT[0:64, i, ts(c, QCH)], av_h[0:64, :], rec[0:64, :]
                    )
                    nc.vector.tensor_mul(
                        oT[64:128, i, ts(c, QCH)], av_p[64:128, :], rec[64:128, :]
                    )

        # ---- phase D: out = O^T.T @ Wo ------------------------------------
        with tc.tile_pool(name="dwo", bufs=1) as dwo, \
             tc.tile_pool(name="dout", bufs=4) as dout, \
             tc.tile_pool(name="dpsum", bufs=4, space="PSUM") as dpsum:
            wo_sb = dwo.tile([128, DT, D], BF16, name="wo_sb")
            nc.gpsimd.dma_start(
                out=wo_sb, in_=wo_ext.rearrange("(a p) n -> p a n", p=128)
            )
            for mq in range(NQ // 128):  # 16 q row blocks
                for n in range(2):
                    ps = dpsum.tile([128, 512], F32, name="ps", tag="ps")
                    for k in range(DT):
                        nc.tensor.matmul(
                            ps[:, :], oT[:, k, ts(mq, 128)],
                            wo_sb[:, k, ts(n, 512)],
                            start=(k == 0), stop=(k == DT - 1),
                        )
                    ob = dout.tile([128, 512], F32, name="ob", tag="ob")
                    nc.vector.tensor_copy(ob, ps)
                    nc.sync.dma_start(
                        out=out_ext[ts(mq, 128), ts(n, 512)], in_=ob
                    )

        # release singles in stack order (right: qT; left: down to ident)
        free_qT()
        free_oT()
        free_vv()
        free_kT()
        free_mask()
        free_ones()
        free_ident()

    nc.finalize()
    return nc


_NC_CACHE = None


def _get_nc():
    global _NC_CACHE
    if _NC_CACHE is None:
        _NC_CACHE = build_nc()
    return _NC_CACHE


def kernel(x, context, context_mask, Wq, Wk, Wv, Wo):
    x = np.ascontiguousarray(np.asarray(x, dtype=np.float32))
    context = np.ascontiguousarray(np.asarray(context, dtype=np.float32))
    Wq = np.ascontiguousarray(np.asarray(Wq, dtype=np.float32))
    Wk = np.ascontiguousarray(np.asarray(Wk, dtype=np.float32))
    Wv = np.ascontiguousarray(np.asarray(Wv, dtype=np.float32))
    Wo = np.ascontiguousarray(np.asarray(Wo, dtype=np.float32))
    mask = np.asarray(context_mask)

    # additive exp-bias per kv position: 0 where visible, -1e9 where masked
    bias = (mask.astype(np.float32) - 1.0) * 1e9          # [B, M]
    x_flat = x.reshape(B * N, D)

    nc = _get_nc()
    in_maps = []
    for c in range(N_CORES):
        b = (c * NQ) // N
        in_maps.append({
            "x": x_flat[c * NQ : (c + 1) * NQ],
            "ctx": context[b],
            "maskb": np.ascontiguousarray(bias[b].reshape(2, 128).T),
            "wq": Wq, "wk": Wk, "wv": Wv, "wo": Wo,
        })
    res = run_bass_kernel_spmd(nc, in_maps, core_ids=list(range(N_CORES)))
    out = np.concatenate([res.results[c]["out"] for c in range(N_CORES)], axis=0)
    return out.reshape(B, N, D)


# revision 11
# speedup vs baseline: 1.6659x; 1.6659x over previous
"""Cross-attention kernel for 8 TRN2 NeuronCores.

Reference computation (per problem spec):
    q = (x @ Wq)  [B=4, N=4096, D=1024] -> heads [B, 16, N, 64]
    k = (context @ Wk), v = (context @ Wv)   context [B, M=256, 768]
    out = softmax(q k^T / 8 + mask) v   -> [B, N, D] @ Wo

Sharding: the 16384 query rows (B*N) are split evenly across the 8 cores
(2048 rows each, each shard living inside one batch). K/V are computed
redundantly per core from that core's batch context (only ~0.8 GFLOP) so no
collectives are needed; each core produces its own 2048 output rows and the
host concatenates them.

On-core dataflow (bf16 matmul operands, fp32 PSUM accumulation), fully
pipelined over 512-row query chunks so TensorE never starves:
  per chunk c: x rows -> x^T (PE transpose) -> Q^T = Wq^T x^T ->
  scores^T = K Q^T (kv on partitions, two heads row-packed in the PE) ->
  exp on ScalarE with the context mask as per-partition bias -> O^T = V^T
  attn^T (two heads col-packed, separate PSUM banks) with softmax sums from
  an all-ones stationary matmul -> normalize on eviction (fast reciprocal)
  -> out rows = O^T.T Wo.
"""

import sys

for _p in ("/opt/trn_rl_repo",):
    if _p not in sys.path:
        sys.path.insert(0, _p)

import numpy as np

import concourse.bass as bass
import concourse.mybir as mybir
import concourse.tile as tile
from concourse import bacc
from concourse.bass_utils import run_bass_kernel_spmd
from concourse.masks import make_identity

ts = bass.ts

N_CORES = 8
B, N, D = 4, 4096, 1024
CTX = 768
M = 256          # kv length
H, HD = 16, 64   # heads, head dim
NQ = (B * N) // N_CORES   # 2048 query rows per core
QCH = 512                 # q chunk (free dim of most matmuls)
NQC = NQ // QCH           # 4 q chunks
DT = D // 128             # 8 d-blocks (= head pairs)
KCH = CTX // 128          # 6 contraction chunks for context projections
F32 = mybir.dt.float32
BF16 = mybir.dt.bfloat16

SCALE = HD ** -0.5


def build_nc():
    nc = bacc.Bacc()

    x_ext = nc.declare_dram_parameter("x", [NQ, D], F32, isOutput=False)
    ctx_ext = nc.declare_dram_parameter("ctx", [M, CTX], F32, isOutput=False)
    maskb_ext = nc.declare_dram_parameter("maskb", [128, 2], F32, isOutput=False)
    wq_ext = nc.declare_dram_parameter("wq", [D, D], F32, isOutput=False)
    wk_ext = nc.declare_dram_parameter("wk", [CTX, D], F32, isOutput=False)
    wv_ext = nc.declare_dram_parameter("wv", [CTX, D], F32, isOutput=False)
    wo_ext = nc.declare_dram_parameter("wo", [D, D], F32, isOutput=False)
    out_ext = nc.declare_dram_parameter("out", [NQ, D], F32, isOutput=True)

    with tile.TileContext(nc) as tc:
        # ---- persistent tensors -------------------------------------------
        identf, free_identf = tc.tile([128, 128], F32, name="identf")
        make_identity(nc, identf)
        identb, free_identb = tc.tile([128, 128], BF16, name="identb")
        make_identity(nc, identb)
        ones_sb, free_ones = tc.tile([128, 128], BF16, name="ones_sb")
        nc.vector.memset(ones_sb, 1.0)
        mask_sb, free_mask = tc.tile([128, 2], F32, name="mask_sb")
        nc.sync.dma_start(out=mask_sb, in_=maskb_ext[:, :])

        kT, free_kT = tc.tile([128, DT, M], BF16, name="kT")
        vv, free_vv = tc.tile([128, 2, D], BF16, name="vv")
        xT, free_xT = tc.tile([128, DT, NQ], BF16, name="xT")
        qT, free_qT = tc.tile([128, DT, NQ], BF16, name="qT")
        oT, free_oT = tc.tile([128, DT, NQ], BF16, name="oT")

        with tc.tile_pool(name="weights", bufs=1) as wpool, \
             tc.tile_pool(name="bpool", bufs=1) as bpool, \
             tc.tile_pool(name="xpool", bufs=4) as xpool, \
             tc.tile_pool(name="attnp", bufs=3) as attnp, \
             tc.tile_pool(name="recp", bufs=2) as recp, \
             tc.tile_pool(name="outp", bufs=3) as outp, \
             tc.tile_pool(name="mpsum", bufs=4, space="PSUM") as mpsum:
            # weight cast-loads on the SWDGE path (f32 -> bf16), in the order
            # the pipeline consumes them
            ctx_sb = bpool.tile([128, 2, CTX], BF16, name="ctx_sb")
            nc.gpsimd.dma_start(
                out=ctx_sb, in_=ctx_ext.rearrange("(a p) n -> p a n", p=128)
            )
            wk_sb = bpool.tile([128, KCH, D], BF16, name="wk_sb")
            nc.gpsimd.dma_start(
                out=wk_sb, in_=wk_ext.rearrange("(a p) n -> p a n", p=128)
            )
            wv_sb = bpool.tile([128, KCH, D], BF16, name="wv_sb")
            nc.gpsimd.dma_start(
                out=wv_sb, in_=wv_ext.rearrange("(a p) n -> p a n", p=128)
            )
            wq_sb = wpool.tile([128, DT, D], BF16, name="wq_sb")
            nc.gpsimd.dma_start(
                out=wq_sb, in_=wq_ext.rearrange("(a p) n -> p a n", p=128)
            )
            wo_sb = wpool.tile([128, DT, D], BF16, name="wo_sb")
            nc.gpsimd.dma_start(
                out=wo_sb, in_=wo_ext.rearrange("(a p) n -> p a n", p=128)
            )

            # ---- K^T / V from context (small; fills PE while x loads) -----
            ctxT = bpool.tile([128, KCH, M], BF16, name="ctxT")
            for a in range(2):  # kv row blocks
                tp = mpsum.tile([128, KCH, 128], BF16, name="tp_b", tag="ps")
                for k in range(KCH):
                    nc.tensor.transpose(
                        tp[:, k, :], ctx_sb[:, a, ts(k, 128)], identb
                    )
                nc.vector.tensor_copy(ctxT[:, :, ts(a, 128)], tp)
            for m in range(DT):
                ps = mpsum.tile([128, M], F32, name="ps_k", tag="ps")
                for k in range(KCH):
                    nc.tensor.matmul(
                        ps[:, :], wk_sb[:, k, ts(m, 128)], ctxT[:, k, :],
                        start=(k == 0), stop=(k == KCH - 1),
                    )
                nc.vector.tensor_copy(kT[:, m, :], ps)
            for j in range(2):
                for n in range(2):
                    psv = mpsum.tile([128, 512], F32, name="ps_v", tag="ps")
                    for k in range(KCH):
                        nc.tensor.matmul(
                            psv[:, :], ctxT[:, k, ts(j, 128)],
                            wv_sb[:, k, ts(n, 512)],
                            start=(k == 0), stop=(k == KCH - 1),
                        )
                    nc.vector.tensor_copy(vv[:, j, ts(n, 512)], psv)

            # ---- main pipeline over q chunks ------------------------------
            for c in range(NQC):
                # x rows 512c..512c+511 -> x^T (f32 loads on the HWDGE path,
                # PE transpose, cast to bf16 on eviction)
                for rr in range(4):
                    rb = 4 * c + rr
                    x_sb = xpool.tile([128, D], F32, name="x_sb", tag="x_sb")
                    nc.sync.dma_start(out=x_sb, in_=x_ext[ts(rb, 128), :])
                    for kg in range(2):
                        tp = mpsum.tile([128, 4, 128], F32, name="tp", tag="ps")
                        for kk in range(4):
                            nc.tensor.transpose(
                                tp[:, kk, :], x_sb[:, ts(4 * kg + kk, 128)],
                                identf,
                            )
                        nc.vector.tensor_copy(
                            xT[:, 4 * kg : 4 * kg + 4, ts(rb, 128)], tp
                        )
                # Q^T chunk c
                for m in range(DT):
                    ps = mpsum.tile([128, QCH], F32, name="ps_q", tag="ps")
                    for k in range(DT):
                        nc.tensor.matmul(
                            ps[:, :], wq_sb[:, k, ts(m, 128)],
                            xT[:, k, ts(c, QCH)],
                            start=(k == 0), stop=(k == DT - 1),
                        )
                    nc.vector.tensor_copy(qT[:, m, ts(c, QCH)], ps)
                # attention chunk c, per head pair
                for i in range(DT):
                    attns = []
                    for j in range(2):  # kv chunk
                        sc_h = mpsum.tile([128, QCH], F32, name="sc_h", tag="ps")
                        sc_p = mpsum.tile([128, QCH], F32, name="sc_p", tag="ps")
                        nc.tensor.matmul(
                            sc_h[:, :], kT[0:64, i, ts(j, 128)],
                            qT[0:64, i, ts(c, QCH)],
                            start=True, stop=True, tile_position=(0, 0),
                        )
                        nc.tensor.matmul(
                            sc_p[:, :], kT[64:128, i, ts(j, 128)],
                            qT[64:128, i, ts(c, QCH)],
                            start=True, stop=True, tile_position=(64, 0),
                        )
                        at_h = attnp.tile([128, QCH], BF16, name="at_h", tag="at_h")
                        at_p = attnp.tile([128, QCH], BF16, name="at_p", tag="at_p")
                        nc.scalar.activation(
                            at_h, sc_h, mybir.ActivationFunctionType.Exp,
                            bias=mask_sb[:, j : j + 1], scale=SCALE,
                        )
                        nc.scalar.activation(
                            at_p, sc_p, mybir.ActivationFunctionType.Exp,
                            bias=mask_sb[:, j : j + 1], scale=SCALE,
                        )
                        attns.append((at_h, at_p))
                    av_h = mpsum.tile([128, QCH], F32, name="av_h", tag="av_h", bufs=1)
                    av_p = mpsum.tile([128, QCH], F32, name="av_p", tag="av_p", bufs=1)
                    sm_h = mpsum.tile([128, QCH], F32, name="sm_h", tag="sm_h", bufs=1)
                    sm_p = mpsum.tile([128, QCH], F32, name="sm_p", tag="sm_p", bufs=1)
                    for j in range(2):
                        at_h, at_p = attns[j]
                        nc.tensor.matmul(
                            av_h[0:64, :], vv[:, j, ts(2 * i, HD)], at_h,
                            start=(j == 0), stop=(j == 1), tile_position=(0, 0),
                        )
                        nc.tensor.matmul(
                            av_p[64:128, :], vv[:, j, ts(2 * i + 1, HD)], at_p,
                            start=(j == 0), stop=(j == 1), tile_position=(0, 64),
                        )
                        nc.tensor.matmul(
                            sm_h[:, :], ones_sb[:, :], at_h,
                            start=(j == 0), stop=(j == 1), tile_position=(0, 0),
                        )
                        nc.tensor.matmul(
                            sm_p[:, :], ones_sb[:, :], at_p,
                            start=(j == 0), stop=(j == 1), tile_position=(0, 0),
                        )
                    rec_h = recp.tile([128, QCH], F32, name="rec_h", tag="rec_h")
                    rec_p = recp.tile([128, QCH], F32, name="rec_p", tag="rec_p")
                    nc.vector.reciprocal_approx_fast(rec_h, sm_h)
                    nc.vector.reciprocal_approx_fast(rec_p, sm_p)
                    nc.vector.tensor_mul(
                        oT[0:64, i, ts(c, QCH)], av_h[0:64, :], rec_h[0:64, :]
                    )
                    nc.vector.tensor_mul(
                        oT[64:128, i, ts(c, QCH)], av_p[64:128, :], rec_p[64:128, :]
                    )
                # out rows chunk c = O^T.T @ Wo
                for mr in range(4):
                    mq = 4 * c + mr
                    for n in range(2):
                        ps = mpsum.tile([128, 512], F32, name="ps_o", tag="ps")
                        for k in range(DT):
                            nc.tensor.matmul(
                                ps[:, :], oT[:, k, ts(mq, 128)],
                                wo_sb[:, k, ts(n, 512)],
                                start=(k == 0), stop=(k == DT - 1),
                            )
                        ob = outp.tile([128, 512], F32, name="ob", tag="ob")
                        nc.vector.tensor_copy(ob, ps)
                        nc.sync.dma_start(
                            out=out_ext[ts(mq, 128), ts(n, 512)], in_=ob
                        )

        # release singles in reverse allocation order
        free_oT()
        free_qT()
        free_xT()
        free_vv()
        free_kT()
        free_mask()
        free_ones()
        free_identb()
        free_identf()

    nc.finalize()
    return nc


_NC_CACHE = None


def _get_nc():
    global _NC_CACHE
    if _NC_CACHE is None:
        _NC_CACHE = build_nc()
    return _NC_CACHE


def kernel(x, context, context_mask, Wq, Wk, Wv, Wo):
    x = np.ascontiguousarray(np.asarray(x, dtype=np.float32))
    context = np.ascontiguousarray(np.asarray(context, dtype=np.float32))
    Wq = np.ascontiguousarray(np.asarray(Wq, dtype=np.float32))
    Wk = np.ascontiguousarray(np.asarray(Wk, dtype=np.float32))
    Wv = np.ascontiguousarray(np.asarray(Wv, dtype=np.float32))
    Wo = np.ascontiguousarray(np.asarray(Wo, dtype=np.float32))
    mask = np.asarray(context_mask)

    # additive exp-bias per kv position: 0 where visible, -1e9 where masked
    bias = (mask.astype(np.float32) - 1.0) * 1e9          # [B, M]
    x_flat = x.reshape(B * N, D)

    nc = _get_nc()
    in_maps = []
    for c in range(N_CORES):
        b = (c * NQ) // N
        in_maps.append({
            "x": x_flat[c * NQ : (c + 1) * NQ],
            "ctx": context[b],
            "maskb": np.ascontiguousarray(bias[b].reshape(2, 128).T),
            "wq": Wq, "wk": Wk, "wv": Wv, "wo": Wo,
        })
    res = run_bass_kernel_spmd(nc, in_maps, core_ids=list(range(N_CORES)))
    out = np.concatenate([res.results[c]["out"] for c in range(N_CORES)], axis=0)
    return out.reshape(B, N, D)
